# revision 19
# baseline (speedup 1.0000x reference)
"""CapsNet routing-by-agreement kernel for 8 TRN2 NeuronCores (v2).

Strategy (in_caps sharded 8-way):
  - Each core owns I_loc = 512 in_caps. Its W shard lives entirely in SBUF
    (two bf16 layouts, 8 MB), so routing iterations do ZERO HBM traffic for
    W / u_hat.  u_hat is never materialized; each routing iteration
    recomputes the two W contractions on the TensorEngine with 128-deep
    packed contractions:
      a-path:  Wv[b,i,o,k] = sum_j W[i,o,j,k] v[b,o,j]   (contract (o8,j)=128,
               block-diag v as stationary operand)
               a[b,i,o]    = sum_k u[b,i,k] Wv[b,i,o,k]  (DVE mul + add-tree)
      s-path:  s[b,o,j]    = sum_{i,k} (c*u)[..] W[..]   (contract i mod 128,
               PSUM-accum over (i-block, k); 8x block-diag fanout over o8,
               diagonal extracted with a constant mask)
  - Cross-core exchange of the partial sums s [64,32,16] happens only for
    routing iterations 1 and 2 (bf16 AllGather, 64 KB per rank, + local
    on-chip sum).  Iteration 3 needs no exchange: each core DMAs its f32
    partial s out, and the host sums the 8 partials and applies the final
    squash in numpy.
  - squash on device is DVE-only: rsqrt via bit-trick seed + 2 Newton
    steps (no Ln/Exp activation-table loads in the squash path).
  - Pipeline granule is (Gq, oc-pair): wv drains land per oc-pair so the
    DVE mul/add-tree for one half starts while the other half's matmuls
    still stream.

Index conventions (per core c): i_glob = c*512 + Gq*128 + p (Gq in 0..3,
p in 0..127);  o = oc*8 + o8 (oc in 0..3);  q = o8*16 + j (j in 0..15).

Host layouts:
  wd  [128,16384] bf16 : wd[q, ((oc*4+Gq)*8+k)*128 + p]  = W[i,o,j,k]
  wb  [128,16384] bf16 : wb[p, ((Gq*8+k)*4+oc)*128 + q]  = W[i,o,j,k]
  uy  [128, 2048] bf16 : uy[p, (Gq*8+k)*64 + b]          = u[b,i,k]
  dlt [128,  128] bf16 : dlt[q, m] = (q//16 == m//16)      (squash sum_j)
  mbd [128, 2048] bf16 : mbd[q, (oc,o8p,b)] = (q//16 == o8p)
  msk [128,  512] f32  : msk[q, (o8p,b)]   = (q//16 == o8p)
Output:
  sout [128, 256] f32 : sout[o8*16+j, oc*64+b] = partial_s[b, oc*8+o8, j]
"""

import os
import sys

import numpy as np
import ml_dtypes

sys.path.insert(0, "/opt/trn_rl_repo")

B, IN_CAPS, IN_DIM = 64, 4096, 8
OUT_CAPS, OUT_DIM = 32, 16
EPS = 1e-8
N_CORES = 8
I_LOC = IN_CAPS // N_CORES  # 512
NG = I_LOC // 128           # 4

_BF16 = ml_dtypes.bfloat16

_CACHE = {}


def _build_program(reps=1, variant="full"):
    import concourse.bass as bass
    import concourse.bacc as bacc
    import concourse.mybir as mybir
    import concourse.tile as tile
    from contextlib import ExitStack

    f32 = mybir.dt.float32
    i32 = mybir.dt.int32
    bf16 = mybir.dt.bfloat16
    vset = set(variant.split(","))
    exch = "noexch" if "noexch" in vset else "ag"
    AF = mybir.ActivationFunctionType
    ALU = mybir.AluOpType

    nc = bacc.Bacc(
        "TRN2",
        target_bir_lowering=False,
        debug=False,
        enable_asserts=False,
        num_devices=N_CORES,
    )

    wd_d = nc.dram_tensor("wd", [128, 16384], bf16, kind="ExternalInput")
    wb_d = nc.dram_tensor("wb", [128, 16384], bf16, kind="ExternalInput")
    uy_d = nc.dram_tensor("uy", [128, 2048], bf16, kind="ExternalInput")
    dlt_d = nc.dram_tensor("dlt", [128, 128], bf16, kind="ExternalInput")
    mbd_d = nc.dram_tensor("mbd", [128, 2048], bf16, kind="ExternalInput")
    msk_d = nc.dram_tensor("msk", [128, 512], f32, kind="ExternalInput")
    sout_d = nc.dram_tensor("sout", [128, 256], f32, kind="ExternalOutput")

    with tile.TileContext(nc) as tc:
        with ExitStack() as ctx:
            sb = ctx.enter_context(tc.tile_pool(name="sb", bufs=1))
            ps_pool_s = ctx.enter_context(
                tc.tile_pool(name="ps_s", bufs=1, space="PSUM"))
            ps_pool_wv = ctx.enter_context(
                tc.tile_pool(name="ps_wv", bufs=2, space="PSUM"))
            dram = ctx.enter_context(tc.tile_pool(name="dram", bufs=2,
                                                  space="DRAM"))

            WD = sb.tile([128, 16384], bf16, tag="WD")
            WBQ = [sb.tile([128, 4096], bf16, tag=f"WB{i}", name=f"WB{i}")
                   for i in range(4)]
            UY = sb.tile([128, 2048], bf16, tag="UY")
            DLT = sb.tile([128, 128], bf16, tag="DLT")
            MBD = sb.tile([128, 2048], bf16, tag="MBD")
            MSK = sb.tile([128, 512], f32, tag="MSK")

            # iteration-1 matmuls consume exactly WBQ[Gq]; order the loads so
            # compute starts as soon as the first quarter lands
            nc.sync.dma_start(UY[:], uy_d[:])
            for i in range(4):
                nc.sync.dma_start(WBQ[i][:], wb_d[:, i * 4096 : (i + 1) * 4096])
            nc.sync.dma_start(DLT[:], dlt_d[:])
            nc.sync.dma_start(MBD[:], mbd_d[:])
            nc.sync.dma_start(MSK[:], msk_d[:])
            nc.sync.dma_start(WD[:], wd_d[:])

            def WBs(idx):
                quarter, off = divmod(idx * 128, 4096)
                return WBQ[quarter][:, off : off + 128]

            # big per-G scratch, parity double-buffered:
            # wv (drains) -> uwv (in-place mul) -> tree partials -> cu
            WVP = [sb.tile([128, 16384], bf16, tag=f"WV{i}", name=f"WV{i}")
                   for i in range(2)]
            BLOG = sb.tile([128, 8192], bf16, tag="BLOG")
            EGP = [sb.tile([128, 2048], bf16, tag=f"EG{i}", name=f"EG{i}")
                   for i in range(2)]
            URC = sb.tile([128, 512], bf16, tag="URC")
            SMT = sb.tile([128, 1024], bf16, tag="SMT")
            MSKD = sb.tile([128, 512], f32, tag="MSKD")
            GAT = sb.tile([128, 2048], bf16, tag="GAT")
            SSB = sb.tile([128, 256], f32, tag="SSB")
            SSBH = sb.tile([128, 256], bf16, tag="SSBH")
            SE = sb.tile([128, 64], f32, tag="SE")
            RCP = sb.tile([128, 64], f32, tag="RCP")
            RCPB = sb.tile([128, 64], bf16, tag="RCPB")
            VBD = sb.tile([128, 2048], bf16, tag="VBD")
            S2 = sb.tile([128, 256], bf16, tag="S2")
            XSS = sb.tile([128, 256], f32, tag="XSS")
            HLF = sb.tile([128, 256], f32, tag="HLF")
            YQ = sb.tile([128, 256], f32, tag="YQ")
            TQ = sb.tile([128, 256], f32, tag="TQ")
            SSP1 = sb.tile([128, 256], f32, tag="SSP1")
            RCP1 = sb.tile([128, 256], f32, tag="RCP1")
            T1 = sb.tile([128, 256], f32, tag="T1")
            SCL = sb.tile([128, 256], f32, tag="SCL")
            VSB = sb.tile([128, 256], bf16, tag="VSB")

            def exchange(t, rep):
                """SSBH partial -> SSB global (bf16 AllGather + local sum)."""
                if exch == "noexch":
                    nc.vector.tensor_copy(SSB[:], SSBH[:])
                    return  # timing-only: skip the cross-core exchange
                ag_in = dram.tile([128, 256], bf16, tag="ag_in",
                                  name=f"ag_in_{rep}_{t}")
                ag_out = dram.tile([1024, 256], bf16, tag="ag_out",
                                   name=f"ag_out_{rep}_{t}")
                nc.sync.dma_start(ag_in[:], SSBH[:])
                nc.gpsimd.collective_compute(
                    "AllGather", ALU.bypass,
                    replica_groups=[list(range(N_CORES))],
                    ins=[ag_in[:].opt()], outs=[ag_out[:].opt()],
                )
                nc.sync.dma_start(
                    GAT[:].rearrange("p (r f) -> p r f", r=8),
                    ag_out[:].rearrange("(r p) f -> p r f", p=128),
                )
                g3 = GAT[:].rearrange("p (r f) -> p r f", r=8)
                nc.vector.tensor_add(g3[:, 0:4], g3[:, 0:4], g3[:, 4:8])
                nc.vector.tensor_add(g3[:, 0:2], g3[:, 0:2], g3[:, 2:4])
                nc.vector.tensor_add(SSB[:], GAT[:, 0:256], GAT[:, 256:512])

            def squash_v(t, rep):
                """SSB (global s) -> v, broadcast into VBD. DVE-only: rsqrt
                via bit-trick seed + 2 Newton steps (no act tables)."""
                nc.vector.tensor_mul(S2[:], SSB[:], SSB[:])
                ps_sq = ps_pool_wv.tile([128, 256], f32, tag="wvp",
                                        name=f"ps_sq_{rep}_{t}")
                nc.tensor.matmul(ps_sq[:], DLT[:], S2[:], start=True, stop=True)
                nc.vector.tensor_scalar_add(XSS[:], ps_sq[:], EPS)
                nc.vector.tensor_scalar_mul(HLF[:], XSS[:], 0.5)
                xi = XSS[:].bitcast(i32)
                yi = YQ[:].bitcast(i32)
                nc.vector.tensor_scalar(yi, xi, 1, None, ALU.arith_shift_right)
                nc.vector.tensor_scalar(yi, yi, -1, 0x5F3759DF,
                                        ALU.mult, ALU.add)
                # one Newton step: seed err ~3.4% -> ~0.2%, below bf16 noise
                for _ in range(1):
                    nc.vector.tensor_mul(TQ[:], YQ[:], YQ[:])
                    nc.vector.tensor_mul(TQ[:], TQ[:], HLF[:])
                    nc.vector.tensor_scalar(TQ[:], TQ[:], -1.0, 1.5,
                                            ALU.mult, ALU.add)
                    nc.vector.tensor_mul(YQ[:], YQ[:], TQ[:])
                nc.vector.tensor_scalar_add(SSP1[:], ps_sq[:], 1.0)
                nc.vector.reciprocal(RCP1[:], SSP1[:])
                nc.vector.tensor_mul(T1[:], RCP1[:], YQ[:])
                nc.vector.tensor_mul(SCL[:], ps_sq[:], T1[:])
                nc.vector.tensor_mul(VSB[:], SSB[:], SCL[:])
                vsb_b = (VSB[:].rearrange("q (oc b) -> q oc b", oc=4)
                         .unsqueeze(2).broadcast_to([128, 4, 8, 64]))
                mbd4 = MBD[:].rearrange("q (oc o8 b) -> q oc o8 b",
                                        oc=4, o8=8)
                vbd4 = VBD[:].rearrange("q (oc o8 b) -> q oc o8 b",
                                        oc=4, o8=8)
                nc.vector.tensor_mul(vbd4, vsb_b, mbd4)

            for rep in range(reps):
                # ---------- iteration 1: s1 = (1/32) sum_i u_hat ----------
                ps_s1 = ps_pool_s.tile([128, 2048], f32, tag="ps_s",
                                       name=f"ps_s0_{rep}")
                for Gq in range(NG):
                    for k in range(8):
                        for oc in range(4):
                            nc.tensor.matmul(
                                ps_s1[:, oc * 512 : oc * 512 + 64],
                                WBs((Gq * 8 + k) * 4 + oc),
                                UY[:, (Gq * 8 + k) * 64 :
                                      (Gq * 8 + k) * 64 + 64],
                                start=(Gq == 0 and k == 0),
                                stop=(Gq == NG - 1 and k == 7),
                            )
                for oc in range(4):
                    nc.scalar.mul(SSBH[:, oc * 64 : oc * 64 + 64],
                                  ps_s1[:, oc * 512 : oc * 512 + 64],
                                  1.0 / 32.0)
                exchange(1, rep)
                squash_v(1, rep)

                # ---------- iterations 2, 3 ----------
                for t in (2, 3):
                    ps_s = ps_pool_s.tile([128, 2048], f32, tag="ps_s",
                                          name=f"ps_s{rep}_{t}")

                    def wv_half(Gq, ocp, ramp=False):
                        WVG = WVP[Gq % 2]
                        for k in range(8):
                            wvp = ps_pool_wv.tile(
                                [128, 1024], f32, tag="wvp",
                                name=f"wvp_{rep}_{t}_{Gq}_{ocp}_{k}")
                            for kk in range(2):
                                oc = ocp * 2 + kk
                                nc.tensor.matmul(
                                    wvp[:, kk * 512 : kk * 512 + 512],
                                    WD[:, ((oc * 4 + Gq) * 8 + k) * 128 :
                                          ((oc * 4 + Gq) * 8 + k) * 128
                                          + 128],
                                    VBD[:, oc * 512 : oc * 512 + 512],
                                    start=True, stop=True,
                                )
                            dst = WVG[:, k * 2048 + ocp * 1024 :
                                         k * 2048 + ocp * 1024 + 1024]
                            # at the iteration-restart ramp DVE is idle, so
                            # split the first chunk's drains across both
                            # engines to shorten the DVE warm-up latency
                            if ramp and k % 2 == 1:
                                nc.vector.tensor_copy(dst, wvp[:])
                            else:
                                nc.scalar.copy(dst, wvp[:])

                    def post_a_half(Gq, ocp, ramp=False):
                        """u*Wv mul + k-add-tree + exp for one oc-pair."""
                        WVG = WVP[Gq % 2]
                        wv4 = WVG[:].rearrange("p (k x b) -> p k x b",
                                               k=8, x=32)
                        wvh = wv4[:, :, ocp * 16 : ocp * 16 + 16]
                        uyh = (UY[:, Gq * 512 : Gq * 512 + 512]
                               .rearrange("p (k b) -> p k b", k=8)
                               .unsqueeze(2).broadcast_to([128, 8, 16, 64]))
                        if "skipmt" in vset:
                            pass
                        elif ramp:
                            # start multiplying as soon as the first k-half
                            # of the drains lands
                            nc.vector.tensor_mul(wvh[:, 0:4], wvh[:, 0:4],
                                                 uyh[:, 0:4])
                            nc.vector.tensor_mul(wvh[:, 4:8], wvh[:, 4:8],
                                                 uyh[:, 4:8])
                        else:
                            nc.vector.tensor_mul(wvh, wvh, uyh)
                        A = WVG[:].rearrange("p (k q) -> p k q", k=8)
                        s = slice(ocp * 1024, ocp * 1024 + 1024)
                        if "skipmt" not in vset:
                            nc.vector.tensor_add(A[:, 0:4, s], A[:, 0:4, s],
                                                 A[:, 4:8, s])
                            nc.vector.tensor_add(A[:, 0:2, s], A[:, 0:2, s],
                                                 A[:, 2:4, s])
                            nc.vector.tensor_add(A[:, 0:1, s], A[:, 0:1, s],
                                                 A[:, 1:2, s])
                        a0 = WVG[:, ocp * 1024 : ocp * 1024 + 1024]
                        bsl = slice(Gq * 2048 + ocp * 1024,
                                    Gq * 2048 + ocp * 1024 + 1024)
                        EG = EGP[Gq % 2]
                        esl = slice(ocp * 1024, ocp * 1024 + 1024)
                        if t == 2:
                            nc.vector.tensor_copy(BLOG[:, bsl], a0)
                            nc.scalar.activation(EG[:, esl], a0, AF.Exp)
                        else:
                            nc.vector.tensor_add(a0, a0, BLOG[:, bsl])
                            nc.scalar.activation(EG[:, esl], a0, AF.Exp)

                    def post_b(Gq):
                        """softmax denom + cu + s-matmuls for one Gq."""
                        WVG = WVP[Gq % 2]
                        EG = EGP[Gq % 2]
                        wvg4 = WVG[:].rearrange("p (k x b) -> p k x b",
                                                k=8, x=32)
                        # sum over o: contiguous halving tree (o is outer)
                        nc.vector.tensor_add(SMT[:], EG[:, 0:1024],
                                             EG[:, 1024:2048])
                        nc.vector.tensor_add(SMT[:, 0:512], SMT[:, 0:512],
                                             SMT[:, 512:1024])
                        nc.vector.tensor_add(SMT[:, 0:256], SMT[:, 0:256],
                                             SMT[:, 256:512])
                        nc.vector.tensor_add(SMT[:, 0:128], SMT[:, 0:128],
                                             SMT[:, 128:256])
                        nc.vector.tensor_add(SE[:], SMT[:, 0:64],
                                             SMT[:, 64:128])
                        nc.vector.reciprocal(RCP[:], SE[:])
                        # on DVE, not ACT: an ACT-queue copy here would sit
                        # between the drain batches and head-block them
                        nc.vector.tensor_copy(RCPB[:], RCP[:])
                        # urc = u * (1/Z): folds softmax denom into cu
                        nc.vector.tensor_mul(
                            URC[:].rearrange("p (k b) -> p k b", k=8),
                            UY[:, Gq * 512 : Gq * 512 + 512]
                               .rearrange("p (k b) -> p k b", k=8),
                            RCPB[:].unsqueeze(1).broadcast_to([128, 8, 64]))
                        # cu = e * urc (into WVG, now dead)
                        egb4 = (EG[:].rearrange("p (x b) -> p x b", x=32)
                                .unsqueeze(1).broadcast_to([128, 8, 32, 64]))
                        urc4 = (URC[:].rearrange("p (k b) -> p k b", k=8)
                                .unsqueeze(2).broadcast_to([128, 8, 32, 64]))
                        if "skipcu" not in vset:
                            # k=7 slice on the otherwise-idle GPSIMD engine:
                            # it finishes before DVE clears k=0..6, so this
                            # trims the DVE critical path for free
                            nc.vector.tensor_mul(wvg4[:, 0:7], egb4[:, 0:7],
                                                 urc4[:, 0:7])
                            nc.gpsimd.tensor_mul(wvg4[:, 7:8], egb4[:, 7:8],
                                                 urc4[:, 7:8])
                        if Gq < NG - 1:
                            for k in range(8):
                                for oc in range(4):
                                    nc.tensor.matmul(
                                        ps_s[:, oc * 512 : oc * 512 + 512],
                                        WBs((Gq * 8 + k) * 4 + oc),
                                        WVG[:, k * 2048 + oc * 512 :
                                               k * 2048 + oc * 512 + 512],
                                        start=(Gq == 0 and k == 0),
                                        stop=False,
                                    )
                        else:
                            # last Gq: oc-outer so each oc's accumulation
                            # closes early and its diag extract overlaps the
                            # remaining oc's matmuls
                            for oc in range(4):
                                for k in range(8):
                                    nc.tensor.matmul(
                                        ps_s[:, oc * 512 : oc * 512 + 512],
                                        WBs((Gq * 8 + k) * 4 + oc),
                                        WVG[:, k * 2048 + oc * 512 :
                                               k * 2048 + oc * 512 + 512],
                                        start=False,
                                        stop=(k == 7),
                                    )
                                nc.vector.tensor_mul(
                                    MSKD[:],
                                    ps_s[:, oc * 512 : oc * 512 + 512],
                                    MSK[:])
                                nc.vector.tensor_reduce(
                                    SSB[:, oc * 64 : oc * 64 + 64],
                                    MSKD[:].rearrange(
                                        "q (o8 b) -> q b o8", o8=8),
                                    axis=mybir.AxisListType.X, op=ALU.add)
                                if t < 3:
                                    nc.vector.tensor_copy(
                                        SSBH[:, oc * 64 : oc * 64 + 64],
                                        SSB[:, oc * 64 : oc * 64 + 64])

                    # software pipeline at (Gq, oc-pair) granularity
                    for Gq in range(NG + 1):
                        if Gq >= 1:
                            post_a_half(Gq - 1, 1)
                        if Gq < NG:
                            wv_half(Gq, 0, ramp=(Gq == 0))
                        if Gq >= 1:
                            post_b(Gq - 1)
                        if Gq < NG:
                            wv_half(Gq, 1)
                            post_a_half(Gq, 0, ramp=(Gq == 0))

                    if t < 3:
                        exchange(t, rep)
                        squash_v(t, rep)
                    else:
                        nc.sync.dma_start(sout_d[:], SSB[:])

    nc.compile()
    return nc


def _host_prep(u, W):
    """Build per-core input maps (all host-side permutes)."""
    in_maps = []
    q = np.arange(128)
    dlt = (q[:, None] // 16 == q[None, :] // 16).astype(_BF16)
    o8p = np.arange(8)
    diag = (q[:, None] // 16 == o8p[None, :])
    mbd = np.ascontiguousarray(
        np.broadcast_to(diag[:, None, :, None], (128, 4, 8, 64))
    ).reshape(128, 2048).astype(_BF16)
    msk = np.ascontiguousarray(
        np.broadcast_to(diag[:, :, None], (128, 8, 64))
    ).reshape(128, 512).astype(np.float32)
    for c in range(N_CORES):
        Ws = np.asarray(W[c * I_LOC : (c + 1) * I_LOC], dtype=np.float32)
        us = np.asarray(u[:, c * I_LOC : (c + 1) * I_LOC, :], dtype=np.float32)
        Wr = Ws.reshape(NG, 128, 4, 8, 16, 8)           # [Gq,p,oc,o8,j,k]
        wd = np.ascontiguousarray(
            Wr.transpose(3, 4, 2, 0, 5, 1)              # [o8,j,oc,Gq,k,p]
        ).reshape(128, 16384).astype(_BF16)
        wb = np.ascontiguousarray(
            Wr.transpose(1, 0, 5, 2, 3, 4)              # [p,Gq,k,oc,o8,j]
        ).reshape(128, 16384).astype(_BF16)
        ur = us.reshape(B, NG, 128, 8)                  # [b,Gq,p,k]
        uy = np.ascontiguousarray(
            ur.transpose(2, 1, 3, 0)                    # [p,Gq,k,b]
        ).reshape(128, 2048).astype(_BF16)
        in_maps.append({"wd": wd, "wb": wb, "uy": uy, "dlt": dlt,
                        "mbd": mbd, "msk": msk})
    return in_maps


def kernel(u, W):
    from concourse.bass_utils import run_bass_kernel_spmd

    if "nc" not in _CACHE:
        _CACHE["nc"] = _build_program(variant="full")
    nc = _CACHE["nc"]

    in_maps = _host_prep(u, W)
    res = run_bass_kernel_spmd(
        nc, in_maps, core_ids=list(range(N_CORES)),
        trace=bool(int(os.environ.get("CAPS_TRACE", "0"))),
    )
    if isinstance(res, tuple):
        results = res[0]
    else:
        _CACHE["last_results"] = res
        results = res.results
    s = np.zeros((128, 256), np.float32)
    for r in results:
        s += np.asarray(r["sout"], np.float32)
    t = s.reshape(8, 16, 4, 64)             # [o8, j, oc, b]
    s_full = np.ascontiguousarray(
        t.transpose(3, 2, 0, 1)).reshape(B, OUT_CAPS, OUT_DIM)
    sq = np.sum(s_full * s_full, axis=-1, keepdims=True)
    v = (sq / (1.0 + sq)) * s_full / np.sqrt(sq + EPS)
    return v.astype(np.float32)


# revision 20
# speedup vs baseline: 1.2299x; 1.2299x over previous
"""CapsNet routing-by-agreement kernel for 8 TRN2 NeuronCores (v2).

Strategy (in_caps sharded 8-way):
  - Each core owns I_loc = 512 in_caps. Its W shard lives entirely in SBUF
    (two bf16 layouts, 8 MB), so routing iterations do ZERO HBM traffic for
    W / u_hat.  u_hat is never materialized; each routing iteration
    recomputes the two W contractions on the TensorEngine with 128-deep
    packed contractions:
      a-path:  Wv[b,i,o,k] = sum_j W[i,o,j,k] v[b,o,j]   (contract (o8,j)=128,
               block-diag v as stationary operand)
               a[b,i,o]    = sum_k u[b,i,k] Wv[b,i,o,k]  (DVE mul + add-tree)
      s-path:  s[b,o,j]    = sum_{i,k} (c*u)[..] W[..]   (contract i mod 128,
               PSUM-accum over (i-block, k); 8x block-diag fanout over o8,
               diagonal extracted with a constant mask)
  - Cross-core exchange of the partial sums s [64,32,16] happens only for
    routing iterations 1 and 2 (bf16 AllGather, 64 KB per rank, + local
    on-chip sum).  Iteration 3 needs no exchange: each core DMAs its f32
    partial s out, and the host sums the 8 partials and applies the final
    squash in numpy.
  - squash on device is DVE-only: rsqrt via bit-trick seed + 2 Newton
    steps (no Ln/Exp activation-table loads in the squash path).
  - Pipeline granule is (Gq, oc-pair): wv drains land per oc-pair so the
    DVE mul/add-tree for one half starts while the other half's matmuls
    still stream.

Index conventions (per core c): i_glob = c*512 + Gq*128 + p (Gq in 0..3,
p in 0..127);  o = oc*8 + o8 (oc in 0..3);  q = o8*16 + j (j in 0..15).

Host layouts:
  wd  [128,16384] bf16 : wd[q, ((oc*4+Gq)*8+k)*128 + p]  = W[i,o,j,k]
  wb  [128,16384] bf16 : wb[p, ((Gq*8+k)*4+oc)*128 + q]  = W[i,o,j,k]
  uy  [128, 2048] bf16 : uy[p, (Gq*8+k)*64 + b]          = u[b,i,k]
  dlt [128,  128] bf16 : dlt[q, m] = (q//16 == m//16)      (squash sum_j)
  mbd [128, 2048] bf16 : mbd[q, (oc,o8p,b)] = (q//16 == o8p)
  msk [128,  512] f32  : msk[q, (o8p,b)]   = (q//16 == o8p)
Output:
  sout [128, 256] f32 : sout[o8*16+j, oc*64+b] = partial_s[b, oc*8+o8, j]
"""

import os
import sys

import numpy as np
import ml_dtypes

sys.path.insert(0, "/opt/trn_rl_repo")

B, IN_CAPS, IN_DIM = 64, 4096, 8
OUT_CAPS, OUT_DIM = 32, 16
EPS = 1e-8
N_CORES = 8
I_LOC = IN_CAPS // N_CORES  # 512
NG = I_LOC // 128           # 4

_BF16 = ml_dtypes.bfloat16

_CACHE = {}


def _build_program(reps=1, variant="full"):
    import concourse.bass as bass
    import concourse.bacc as bacc
    import concourse.mybir as mybir
    import concourse.tile as tile
    from contextlib import ExitStack

    f32 = mybir.dt.float32
    i32 = mybir.dt.int32
    bf16 = mybir.dt.bfloat16
    vset = set(variant.split(","))
    exch = "noexch" if "noexch" in vset else "ag"
    AF = mybir.ActivationFunctionType
    ALU = mybir.AluOpType

    nc = bacc.Bacc(
        "TRN2",
        target_bir_lowering=False,
        debug=False,
        enable_asserts=False,
        num_devices=N_CORES,
    )

    wd_d = nc.dram_tensor("wd", [128, 16384], bf16, kind="ExternalInput")
    wb_d = nc.dram_tensor("wb", [128, 16384], bf16, kind="ExternalInput")
    uy_d = nc.dram_tensor("uy", [128, 2048], bf16, kind="ExternalInput")
    dlt_d = nc.dram_tensor("dlt", [128, 128], bf16, kind="ExternalInput")
    mbd_d = nc.dram_tensor("mbd", [128, 2048], bf16, kind="ExternalInput")
    msk_d = nc.dram_tensor("msk", [128, 512], f32, kind="ExternalInput")
    sout_d = nc.dram_tensor("sout", [128, 256], f32, kind="ExternalOutput")

    with tile.TileContext(nc) as tc:
        with ExitStack() as ctx:
            sb = ctx.enter_context(tc.tile_pool(name="sb", bufs=1))
            ps_pool_s = ctx.enter_context(
                tc.tile_pool(name="ps_s", bufs=1, space="PSUM"))
            ps_pool_wv = ctx.enter_context(
                tc.tile_pool(name="ps_wv", bufs=2, space="PSUM"))
            dram = ctx.enter_context(tc.tile_pool(name="dram", bufs=2,
                                                  space="DRAM"))

            WD = sb.tile([128, 16384], bf16, tag="WD")
            WBQ = [sb.tile([128, 4096], bf16, tag=f"WB{i}", name=f"WB{i}")
                   for i in range(4)]
            UY = sb.tile([128, 2048], bf16, tag="UY")
            DLT = sb.tile([128, 128], bf16, tag="DLT")
            MBD = sb.tile([128, 2048], bf16, tag="MBD")
            MSK = sb.tile([128, 512], f32, tag="MSK")

            # iteration-1 matmuls consume exactly WBQ[Gq]; order the loads so
            # compute starts as soon as the first quarter lands
            nc.sync.dma_start(UY[:], uy_d[:])
            for i in range(4):
                nc.sync.dma_start(WBQ[i][:], wb_d[:, i * 4096 : (i + 1) * 4096])
            nc.sync.dma_start(DLT[:], dlt_d[:])
            nc.sync.dma_start(MBD[:], mbd_d[:])
            nc.sync.dma_start(MSK[:], msk_d[:])
            nc.sync.dma_start(WD[:], wd_d[:])

            def WBs(idx):
                quarter, off = divmod(idx * 128, 4096)
                return WBQ[quarter][:, off : off + 128]

            # big per-G scratch, parity double-buffered:
            # wv (drains) -> uwv (in-place mul) -> tree partials -> cu
            WVP = [sb.tile([128, 16384], bf16, tag=f"WV{i}", name=f"WV{i}")
                   for i in range(2)]
            BLOG = sb.tile([128, 8192], bf16, tag="BLOG")
            EGP = [sb.tile([128, 2048], bf16, tag=f"EG{i}", name=f"EG{i}")
                   for i in range(2)]
            URC = sb.tile([128, 512], bf16, tag="URC")
            SMT = sb.tile([128, 1024], bf16, tag="SMT")
            MSKD = sb.tile([128, 512], f32, tag="MSKD")
            GAT = sb.tile([128, 2048], bf16, tag="GAT")
            SSB = sb.tile([128, 256], f32, tag="SSB")
            SSBH = sb.tile([128, 256], bf16, tag="SSBH")
            SE = sb.tile([128, 64], f32, tag="SE")
            RCP = sb.tile([128, 64], f32, tag="RCP")
            RCPB = sb.tile([128, 64], bf16, tag="RCPB")
            VBD = sb.tile([128, 2048], bf16, tag="VBD")
            S2 = sb.tile([128, 256], bf16, tag="S2")
            XSS = sb.tile([128, 256], f32, tag="XSS")
            HLF = sb.tile([128, 256], f32, tag="HLF")
            YQ = sb.tile([128, 256], f32, tag="YQ")
            TQ = sb.tile([128, 256], f32, tag="TQ")
            SSP1 = sb.tile([128, 256], f32, tag="SSP1")
            RCP1 = sb.tile([128, 256], f32, tag="RCP1")
            T1 = sb.tile([128, 256], f32, tag="T1")
            SCL = sb.tile([128, 256], f32, tag="SCL")
            VSB = sb.tile([128, 256], bf16, tag="VSB")

            def exchange(t, rep):
                """SSBH partial -> SSB global (bf16 AllGather + local sum)."""
                if exch == "noexch":
                    nc.vector.tensor_copy(SSB[:], SSBH[:])
                    return  # timing-only: skip the cross-core exchange
                ag_in = dram.tile([128, 256], bf16, tag="ag_in",
                                  name=f"ag_in_{rep}_{t}")
                ag_out = dram.tile([1024, 256], bf16, tag="ag_out",
                                   name=f"ag_out_{rep}_{t}")
                nc.sync.dma_start(ag_in[:], SSBH[:])
                nc.gpsimd.collective_compute(
                    "AllGather", ALU.bypass,
                    replica_groups=[list(range(N_CORES))],
                    ins=[ag_in[:].opt()], outs=[ag_out[:].opt()],
                )
                nc.sync.dma_start(
                    GAT[:].rearrange("p (r f) -> p r f", r=8),
                    ag_out[:].rearrange("(r p) f -> p r f", p=128),
                )
                g3 = GAT[:].rearrange("p (r f) -> p r f", r=8)
                nc.vector.tensor_add(g3[:, 0:4], g3[:, 0:4], g3[:, 4:8])
                nc.vector.tensor_add(g3[:, 0:2], g3[:, 0:2], g3[:, 2:4])
                nc.vector.tensor_add(SSB[:], GAT[:, 0:256], GAT[:, 256:512])

            def squash_v(t, rep):
                """SSB (global s) -> v, broadcast into VBD. DVE-only: rsqrt
                via bit-trick seed + 2 Newton steps (no act tables)."""
                nc.vector.tensor_mul(S2[:], SSB[:], SSB[:])
                ps_sq = ps_pool_wv.tile([128, 256], f32, tag="wvp",
                                        name=f"ps_sq_{rep}_{t}")
                nc.tensor.matmul(ps_sq[:], DLT[:], S2[:], start=True, stop=True)
                nc.vector.tensor_scalar_add(XSS[:], ps_sq[:], EPS)
                nc.vector.tensor_scalar_mul(HLF[:], XSS[:], 0.5)
                xi = XSS[:].bitcast(i32)
                yi = YQ[:].bitcast(i32)
                nc.vector.tensor_scalar(yi, xi, 1, None, ALU.arith_shift_right)
                nc.vector.tensor_scalar(yi, yi, -1, 0x5F3759DF,
                                        ALU.mult, ALU.add)
                # one Newton step: seed err ~3.4% -> ~0.2%, below bf16 noise
                for _ in range(1):
                    nc.vector.tensor_mul(TQ[:], YQ[:], YQ[:])
                    nc.vector.tensor_mul(TQ[:], TQ[:], HLF[:])
                    nc.vector.tensor_scalar(TQ[:], TQ[:], -1.0, 1.5,
                                            ALU.mult, ALU.add)
                    nc.vector.tensor_mul(YQ[:], YQ[:], TQ[:])
                nc.vector.tensor_scalar_add(SSP1[:], ps_sq[:], 1.0)
                nc.vector.reciprocal(RCP1[:], SSP1[:])
                nc.vector.tensor_mul(T1[:], RCP1[:], YQ[:])
                nc.vector.tensor_mul(SCL[:], ps_sq[:], T1[:])
                nc.vector.tensor_mul(VSB[:], SSB[:], SCL[:])
                vsb_b = (VSB[:].rearrange("q (oc b) -> q oc b", oc=4)
                         .unsqueeze(2).broadcast_to([128, 4, 8, 64]))
                mbd4 = MBD[:].rearrange("q (oc o8 b) -> q oc o8 b",
                                        oc=4, o8=8)
                vbd4 = VBD[:].rearrange("q (oc o8 b) -> q oc o8 b",
                                        oc=4, o8=8)
                nc.vector.tensor_mul(vbd4, vsb_b, mbd4)

            for rep in range(reps):
                # ---------- iteration 1: s1 = (1/32) sum_i u_hat ----------
                ps_s1 = ps_pool_s.tile([128, 2048], f32, tag="ps_s",
                                       name=f"ps_s0_{rep}")
                for Gq in range(NG):
                    for k in range(8):
                        for oc in range(4):
                            nc.tensor.matmul(
                                ps_s1[:, oc * 512 : oc * 512 + 64],
                                WBs((Gq * 8 + k) * 4 + oc),
                                UY[:, (Gq * 8 + k) * 64 :
                                      (Gq * 8 + k) * 64 + 64],
                                start=(Gq == 0 and k == 0),
                                stop=(Gq == NG - 1 and k == 7),
                            )
                for oc in range(4):
                    nc.scalar.mul(SSBH[:, oc * 64 : oc * 64 + 64],
                                  ps_s1[:, oc * 512 : oc * 512 + 64],
                                  1.0 / 32.0)
                exchange(1, rep)
                squash_v(1, rep)

                # ---------- iterations 2, 3 ----------
                for t in (2, 3):
                    ps_s = ps_pool_s.tile([128, 2048], f32, tag="ps_s",
                                          name=f"ps_s{rep}_{t}")

                    def wv_half(Gq, ocp, ramp=False):
                        WVG = WVP[Gq % 2]
                        for k in range(8):
                            wvp = ps_pool_wv.tile(
                                [128, 1024], f32, tag="wvp",
                                name=f"wvp_{rep}_{t}_{Gq}_{ocp}_{k}")
                            for kk in range(2):
                                oc = ocp * 2 + kk
                                nc.tensor.matmul(
                                    wvp[:, kk * 512 : kk * 512 + 512],
                                    WD[:, ((oc * 4 + Gq) * 8 + k) * 128 :
                                          ((oc * 4 + Gq) * 8 + k) * 128
                                          + 128],
                                    VBD[:, oc * 512 : oc * 512 + 512],
                                    start=True, stop=True,
                                )
                            dst = WVG[:, k * 2048 + ocp * 1024 :
                                         k * 2048 + ocp * 1024 + 1024]
                            # at the iteration-restart ramp DVE is idle, so
                            # split the first chunk's drains across both
                            # engines to shorten the DVE warm-up latency
                            if ramp and k % 2 == 1:
                                nc.vector.tensor_copy(dst, wvp[:])
                            else:
                                nc.scalar.copy(dst, wvp[:])

                    def post_a_half(Gq, ocp, ramp=False):
                        """u*Wv mul + k-add-tree + exp for one oc-pair."""
                        WVG = WVP[Gq % 2]
                        wv4 = WVG[:].rearrange("p (k x b) -> p k x b",
                                               k=8, x=32)
                        wvh = wv4[:, :, ocp * 16 : ocp * 16 + 16]
                        uyh = (UY[:, Gq * 512 : Gq * 512 + 512]
                               .rearrange("p (k b) -> p k b", k=8)
                               .unsqueeze(2).broadcast_to([128, 8, 16, 64]))
                        if "skipmt" in vset:
                            pass
                        elif ramp:
                            # start multiplying as soon as the first k-half
                            # of the drains lands
                            nc.vector.tensor_mul(wvh[:, 0:4], wvh[:, 0:4],
                                                 uyh[:, 0:4])
                            nc.vector.tensor_mul(wvh[:, 4:8], wvh[:, 4:8],
                                                 uyh[:, 4:8])
                        else:
                            nc.vector.tensor_mul(wvh[:, 0:7], wvh[:, 0:7],
                                                 uyh[:, 0:7])
                            nc.gpsimd.tensor_mul(wvh[:, 7:8], wvh[:, 7:8],
                                                 uyh[:, 7:8])
                        A = WVG[:].rearrange("p (k q) -> p k q", k=8)
                        s = slice(ocp * 1024, ocp * 1024 + 1024)
                        if "skipmt" not in vset:
                            nc.vector.tensor_add(A[:, 0:4, s], A[:, 0:4, s],
                                                 A[:, 4:8, s])
                            nc.vector.tensor_add(A[:, 0:2, s], A[:, 0:2, s],
                                                 A[:, 2:4, s])
                            nc.vector.tensor_add(A[:, 0:1, s], A[:, 0:1, s],
                                                 A[:, 1:2, s])
                        a0 = WVG[:, ocp * 1024 : ocp * 1024 + 1024]
                        bsl = slice(Gq * 2048 + ocp * 1024,
                                    Gq * 2048 + ocp * 1024 + 1024)
                        EG = EGP[Gq % 2]
                        esl = slice(ocp * 1024, ocp * 1024 + 1024)
                        if t == 2:
                            nc.vector.tensor_copy(BLOG[:, bsl], a0)
                            nc.scalar.activation(EG[:, esl], a0, AF.Exp)
                        else:
                            nc.vector.tensor_add(a0, a0, BLOG[:, bsl])
                            nc.scalar.activation(EG[:, esl], a0, AF.Exp)

                    def post_b(Gq):
                        """softmax denom + cu + s-matmuls for one Gq."""
                        WVG = WVP[Gq % 2]
                        EG = EGP[Gq % 2]
                        wvg4 = WVG[:].rearrange("p (k x b) -> p k x b",
                                                k=8, x=32)
                        # sum over o: contiguous halving tree (o is outer)
                        nc.vector.tensor_add(SMT[:], EG[:, 0:1024],
                                             EG[:, 1024:2048])
                        nc.vector.tensor_add(SMT[:, 0:512], SMT[:, 0:512],
                                             SMT[:, 512:1024])
                        nc.vector.tensor_add(SMT[:, 0:256], SMT[:, 0:256],
                                             SMT[:, 256:512])
                        nc.vector.tensor_add(SMT[:, 0:128], SMT[:, 0:128],
                                             SMT[:, 128:256])
                        nc.vector.tensor_add(SE[:], SMT[:, 0:64],
                                             SMT[:, 64:128])
                        nc.vector.reciprocal(RCP[:], SE[:])
                        # on DVE, not ACT: an ACT-queue copy here would sit
                        # between the drain batches and head-block them
                        nc.vector.tensor_copy(RCPB[:], RCP[:])
                        # urc = u * (1/Z): folds softmax denom into cu
                        nc.vector.tensor_mul(
                            URC[:].rearrange("p (k b) -> p k b", k=8),
                            UY[:, Gq * 512 : Gq * 512 + 512]
                               .rearrange("p (k b) -> p k b", k=8),
                            RCPB[:].unsqueeze(1).broadcast_to([128, 8, 64]))
                        # cu = e * urc (into WVG, now dead)
                        egb4 = (EG[:].rearrange("p (x b) -> p x b", x=32)
                                .unsqueeze(1).broadcast_to([128, 8, 32, 64]))
                        urc4 = (URC[:].rearrange("p (k b) -> p k b", k=8)
                                .unsqueeze(2).broadcast_to([128, 8, 32, 64]))
                        if "skipcu" not in vset:
                            # k=7 slice on the otherwise-idle GPSIMD engine:
                            # it finishes before DVE clears k=0..6, so this
                            # trims the DVE critical path for free
                            nc.vector.tensor_mul(wvg4[:, 0:7], egb4[:, 0:7],
                                                 urc4[:, 0:7])
                            nc.gpsimd.tensor_mul(wvg4[:, 7:8], egb4[:, 7:8],
                                                 urc4[:, 7:8])
                        if Gq < NG - 1:
                            for k in range(8):
                                for oc in range(4):
                                    nc.tensor.matmul(
                                        ps_s[:, oc * 512 : oc * 512 + 512],
                                        WBs((Gq * 8 + k) * 4 + oc),
                                        WVG[:, k * 2048 + oc * 512 :
                                               k * 2048 + oc * 512 + 512],
                                        start=(Gq == 0 and k == 0),
                                        stop=False,
                                    )
                        else:
                            # last Gq: oc-outer so each oc's accumulation
                            # closes early and its diag extract overlaps the
                            # remaining oc's matmuls
                            for oc in range(4):
                                for k in range(8):
                                    nc.tensor.matmul(
                                        ps_s[:, oc * 512 : oc * 512 + 512],
                                        WBs((Gq * 8 + k) * 4 + oc),
                                        WVG[:, k * 2048 + oc * 512 :
                                               k * 2048 + oc * 512 + 512],
                                        start=False,
                                        stop=(k == 7),
                                    )
                                nc.vector.tensor_mul(
                                    MSKD[:],
                                    ps_s[:, oc * 512 : oc * 512 + 512],
                                    MSK[:])
                                nc.vector.tensor_reduce(
                                    SSB[:, oc * 64 : oc * 64 + 64],
                                    MSKD[:].rearrange(
                                        "q (o8 b) -> q b o8", o8=8),
                                    axis=mybir.AxisListType.X, op=ALU.add)
                                if t < 3:
                                    nc.vector.tensor_copy(
                                        SSBH[:, oc * 64 : oc * 64 + 64],
                                        SSB[:, oc * 64 : oc * 64 + 64])

                    # software pipeline at (Gq, oc-pair) granularity
                    for Gq in range(NG + 1):
                        if Gq >= 1:
                            post_a_half(Gq - 1, 1)
                        if Gq < NG:
                            wv_half(Gq, 0, ramp=(Gq == 0))
                        if Gq >= 1:
                            post_b(Gq - 1)
                        if Gq < NG:
                            wv_half(Gq, 1)
                            post_a_half(Gq, 0, ramp=(Gq == 0))

                    if t < 3:
                        exchange(t, rep)
                        squash_v(t, rep)
                    else:
                        nc.sync.dma_start(sout_d[:], SSB[:])

    nc.compile()
    return nc


def _host_prep(u, W):
    """Build per-core input maps (all host-side permutes)."""
    in_maps = []
    q = np.arange(128)
    dlt = (q[:, None] // 16 == q[None, :] // 16).astype(_BF16)
    o8p = np.arange(8)
    diag = (q[:, None] // 16 == o8p[None, :])
    mbd = np.ascontiguousarray(
        np.broadcast_to(diag[:, None, :, None], (128, 4, 8, 64))
    ).reshape(128, 2048).astype(_BF16)
    msk = np.ascontiguousarray(
        np.broadcast_to(diag[:, :, None], (128, 8, 64))
    ).reshape(128, 512).astype(np.float32)
    for c in range(N_CORES):
        Ws = np.asarray(W[c * I_LOC : (c + 1) * I_LOC], dtype=np.float32)
        us = np.asarray(u[:, c * I_LOC : (c + 1) * I_LOC, :], dtype=np.float32)
        Wr = Ws.reshape(NG, 128, 4, 8, 16, 8)           # [Gq,p,oc,o8,j,k]
        wd = np.ascontiguousarray(
            Wr.transpose(3, 4, 2, 0, 5, 1)              # [o8,j,oc,Gq,k,p]
        ).reshape(128, 16384).astype(_BF16)
        wb = np.ascontiguousarray(
            Wr.transpose(1, 0, 5, 2, 3, 4)              # [p,Gq,k,oc,o8,j]
        ).reshape(128, 16384).astype(_BF16)
        ur = us.reshape(B, NG, 128, 8)                  # [b,Gq,p,k]
        uy = np.ascontiguousarray(
            ur.transpose(2, 1, 3, 0)                    # [p,Gq,k,b]
        ).reshape(128, 2048).astype(_BF16)
        in_maps.append({"wd": wd, "wb": wb, "uy": uy, "dlt": dlt,
                        "mbd": mbd, "msk": msk})
    return in_maps


def kernel(u, W):
    from concourse.bass_utils import run_bass_kernel_spmd

    if "nc" not in _CACHE:
        _CACHE["nc"] = _build_program(variant="full")
    nc = _CACHE["nc"]

    in_maps = _host_prep(u, W)
    res = run_bass_kernel_spmd(
        nc, in_maps, core_ids=list(range(N_CORES)),
        trace=bool(int(os.environ.get("CAPS_TRACE", "0"))),
    )
    if isinstance(res, tuple):
        results = res[0]
    else:
        _CACHE["last_results"] = res
        results = res.results
    s = np.zeros((128, 256), np.float32)
    for r in results:
        s += np.asarray(r["sout"], np.float32)
    t = s.reshape(8, 16, 4, 64)             # [o8, j, oc, b]
    s_full = np.ascontiguousarray(
        t.transpose(3, 2, 0, 1)).reshape(B, OUT_CAPS, OUT_DIM)
    sq = np.sum(s_full * s_full, axis=-1, keepdims=True)
    v = (sq / (1.0 + sq)) * s_full / np.sqrt(sq + EPS)
    return v.astype(np.float32)
